# revision 1
# baseline (speedup 1.0000x reference)
"""DGCNN (4x DynamicEdgeConv + lin1 + global max pool + MLP head + log_softmax)
Trainium2 Bass kernel, data-parallel over 8 graphs on 8 NeuronCores.

Per core (one graph, N=2048 points):
  Each edge conv layer l (D_in -> D_out):
    msg_ij = [x_i, x_j - x_i] @ W + b = x_i @ (Wa-Wb) + b + x_j @ Wb = p_i + q_j
    out_i  = p_i + max_{j in kNN20(i)} q_j
  kNN via S = <x_i,x_j> - sq_i/2 - sq_j/2 = -d2/2 (TensorE, f32r full rate),
  top-20 per row via index-packing ((S & ~0x7FF) | col_idx) + 3 rounds of
  DVE max8/match_replace, neighbor rows gathered by indirect DMA from a
  per-layer q table in DRAM, 20-way max via one strided DVE reduce.
  Activations kept transposed ([D, 2048]); p-add + bias + transpose of the
  aggregated rows are fused into one PSUM accumulation group on TensorE.
"""
import os

os.environ.setdefault("MYCRO_LOCAL_CACHE", "1")

import numpy as np

import concourse.bass as bass
import concourse.bacc as bacc
import concourse.mybir as mybir
import concourse.tile as tile
from concourse.bass_utils import run_bass_kernel_spmd
from concourse.masks import make_identity
from concourse.library_config import mlp as MLP_LIB

N = 2048  # points per graph
K = 20  # kNN neighbors
P = 128  # partitions
B = 8  # graphs == cores
NT = N // P  # 16 point tiles per graph

f32 = mybir.dt.float32
f32r = mybir.dt.float32r
i32 = mybir.dt.int32

# (D_in, D_out) per edge conv layer
CONV_DIMS = [(3, 64), (64, 64), (64, 128), (128, 256)]
CAT = 512  # 64+64+128+256
LIN1 = 1024
HEAD = [(1024, 512), (512, 256), (256, 10)]

_cache = {}

# precision experiment knobs (read at build time)
F32R_S = os.environ.get("F32R_S", "1") == "1"
F32R_LIN1 = os.environ.get("F32R_LIN1", "1") == "1"
GATHER_MODE = os.environ.get("GATHER_MODE", "indirect")


def _ceil_div(a, b):
    return (a + b - 1) // b


def _build():
    nc = bacc.Bacc("TRN2", target_bir_lowering=False)

    pos = nc.dram_tensor("pos", [N, 3], f32, kind="ExternalInput")
    wts = {}
    for li, (di, do) in enumerate(CONV_DIMS):
        wts[f"w{li + 1}"] = nc.dram_tensor(f"w{li + 1}", [2 * di, do], f32, kind="ExternalInput")
        wts[f"b{li + 1}"] = nc.dram_tensor(f"b{li + 1}", [do], f32, kind="ExternalInput")
    lw = nc.dram_tensor("lw", [CAT, LIN1], f32, kind="ExternalInput")
    lb = nc.dram_tensor("lb", [LIN1], f32, kind="ExternalInput")
    for hi, (di, do) in enumerate(HEAD):
        wts[f"m{hi + 1}w"] = nc.dram_tensor(f"m{hi + 1}w", [di, do], f32, kind="ExternalInput")
        wts[f"m{hi + 1}b"] = nc.dram_tensor(f"m{hi + 1}b", [do], f32, kind="ExternalInput")
    out = nc.dram_tensor("out", [1, 10], f32, kind="ExternalOutput")
    dbg = None
    if os.environ.get("KERNEL_DEBUG"):
        dbg = {
            "dbg_x1": nc.dram_tensor("dbg_x1", [64, N], f32, kind="ExternalOutput"),
            "dbg_x2": nc.dram_tensor("dbg_x2", [64, N], f32, kind="ExternalOutput"),
            "dbg_x3": nc.dram_tensor("dbg_x3", [P, N], f32, kind="ExternalOutput"),
            "dbg_x4a": nc.dram_tensor("dbg_x4a", [P, N], f32, kind="ExternalOutput"),
            "dbg_x4b": nc.dram_tensor("dbg_x4b", [P, N], f32, kind="ExternalOutput"),
            "dbg_g": nc.dram_tensor("dbg_g", [P, 8], f32, kind="ExternalOutput"),
        }

    with tile.TileContext(nc) as tc:
        _emit(nc, tc, pos, wts, lw, lb, out, dbg)
    nc.compile()
    return nc


def _emit(nc, tc, pos, wts, lw, lb, out, dbg=None):
    from contextlib import ExitStack

    AluOp = mybir.AluOpType
    Act = mybir.ActivationFunctionType

    with ExitStack() as ctx:
        const = ctx.enter_context(tc.tile_pool(name="const", bufs=1))
        dram = ctx.enter_context(tc.tile_pool(name="dram", bufs=1, space="DRAM"))
        xpool = ctx.enter_context(tc.tile_pool(name="xt", bufs=1))

        ident = const.tile([P, P], f32)
        make_identity(nc, ident[:])
        iota_t = const.tile([P, N], i32)
        nc.gpsimd.iota(iota_t[:], pattern=[[1, N]], base=0, channel_multiplier=0)
        if GATHER_MODE == "dmagather":
            nc.gpsimd.load_library(MLP_LIB)
        ones_tmp = const.tile([1, 512], f32)
        nc.vector.memset(ones_tmp[:], 1.0)
        ones_row = const.tile([1, 512], f32)
        nc.scalar.copy(out=ones_row[:].bitcast(f32r), in_=ones_tmp[:])
        ones_col = const.tile([P, 1], f32)
        nc.vector.memset(ones_col[:], 1.0)
        mask_col = const.tile([P, 1], i32)
        nc.vector.memset(mask_col[:], -2048)

        # ---- conv weights: wa/wb separated, wp = wa - wb, bias rows
        wp_t, wb_t, br_t = [], [], []
        for li, (di, do) in enumerate(CONV_DIMS):
            w = wts[f"w{li + 1}"]
            wab = const.tile([di, 2, do], f32, tag=f"wab{li}")
            nc.sync.dma_start(
                out=wab[:], in_=w[:].rearrange("(two p) d -> p two d", two=2)
            )
            wb = wab[:, 1, :]
            wp = const.tile([di, do], f32, tag=f"wp{li}")
            nc.vector.tensor_sub(wp[:], wab[:, 0, :], wab[:, 1, :])
            brow = const.tile([1, do], f32, tag=f"br{li}")
            nc.sync.dma_start(out=brow[:], in_=wts[f"b{li + 1}"][None, :])
            wp_t.append(wp)
            wb_t.append(wb)
            br_t.append(brow)

        # ---- x tiles (transposed activations), kept for lin1
        x1T = xpool.tile([64, N], f32)
        x2T = xpool.tile([64, N], f32)
        x3T = xpool.tile([P, N], f32)
        x4Ta = xpool.tile([P, N], f32)
        x4Tb = xpool.tile([P, N], f32)
        x0T = xpool.tile([3, N], f32)

        # initial transpose pos [2048,3] -> x0T [3, 2048]
        with tc.tile_pool(name="ld", bufs=1) as ldp, tc.tile_pool(
            name="ldps", bufs=2, space="PSUM"
        ) as ldps:
            xrows = ldp.tile([P, NT, 3], f32)
            nc.sync.dma_start(out=xrows[:], in_=pos[:].rearrange("(t p) d -> p t d", p=P))
            for t in range(NT):
                pt = ldps.tile([3, P], f32, tag="tp")
                nc.tensor.transpose(pt[:], xrows[:, t, :], ident[:])
                nc.scalar.copy(out=x0T[:, t * P : (t + 1) * P].bitcast(f32r), in_=pt[:])

        def xt_in(li):
            # input activation tiles for layer li as list of (tensor, psl) K-chunks
            if li == 0:
                return [(x0T, slice(0, 3))]
            if li == 1:
                return [(x1T, slice(0, 64))]
            if li == 2:
                return [(x2T, slice(0, 64))]
            if li == 3:
                return [(x3T, slice(0, P))]
            raise ValueError(li)

        def xt_out(li):
            # output tiles for layer li: list of (tensor, out_dim_slice)
            if li == 0:
                return [(x1T, slice(0, 64))]
            if li == 1:
                return [(x2T, slice(0, 64))]
            if li == 2:
                return [(x3T, slice(0, P))]
            if li == 3:
                return [(x4Ta, slice(0, P)), (x4Tb, slice(P, 256))]
            raise ValueError(li)

        # ---------------- edge conv layers ----------------
        with ExitStack() as lctx:
            rows = lctx.enter_context(tc.tile_pool(name="rows", bufs=1))
            work = lctx.enter_context(tc.tile_pool(name="work", bufs=2))
            packp = lctx.enter_context(tc.tile_pool(name="pack", bufs=2))
            qgp = lctx.enter_context(tc.tile_pool(name="qg", bufs=2))
            ps_s = lctx.enter_context(tc.tile_pool(name="ps_s", bufs=1, space="PSUM"))
            ps_q = lctx.enter_context(tc.tile_pool(name="ps_q", bufs=2, space="PSUM"))
            ps_p = lctx.enter_context(tc.tile_pool(name="ps_p", bufs=2, space="PSUM"))

            for li, (di, do) in enumerate(CONV_DIMS):
                xin = xt_in(li)
                q_dram = dram.tile([N, do], f32, tag=f"qd{li}")

                # xsq / msqrow
                xsq = rows.tile([P, N], f32, tag="xsq")
                msqrow = rows.tile([1, N], f32, tag="msqrow")
                for xt_s, psl in xin:
                    nc.scalar.square(xsq[0 : psl.stop - psl.start, :], xt_s[psl, :])
                dsum = sum(psl.stop - psl.start for _, psl in xin)
                for c in range(4):
                    sl = slice(c * 512, (c + 1) * 512)
                    spt = ps_q.tile([1, 512], f32, tag="q")
                    nc.tensor.matmul(
                        spt[:], lhsT=ones_col[0:dsum, :], rhs=xsq[0:dsum, sl],
                        start=True, stop=True,
                    )
                    # msqrow = -sq/2 (fused into the PSUM->SBUF copy)
                    nc.scalar.activation(
                        out=msqrow[:, sl].bitcast(f32r), in_=spt[:],
                        func=Act.Copy, scale=-0.5,
                    )

                # q tiles -> q_dram (row layout), before the S/topk stream
                for t in range(NT):
                    tsl = slice(t * P, (t + 1) * P)
                    qps = ps_q.tile([P, do], f32, tag="q")
                    for ki, (xt_s, psl) in enumerate(xin):
                        nc.tensor.matmul(
                            qps[:], lhsT=xt_s[psl, tsl], rhs=wb_t[li],
                            start=(ki == 0), stop=(ki == len(xin) - 1),
                        )
                    qsb = work.tile([P, do], f32, tag="qsb")
                    nc.scalar.copy(out=qsb[:], in_=qps[:])
                    nc.sync.dma_start(out=q_dram[tsl, :], in_=qsb[:])

                # tail for tile t: 20-way max reduce + fused p/bias/transpose
                def emit_tail(t, qg, li=li, do=do, xin=xin):
                    tsl = slice(t * P, (t + 1) * P)
                    agg = work.tile([P, do], f32, tag="agg")
                    nc.vector.tensor_reduce(
                        out=agg[:], in_=qg[:].rearrange("p k d -> p d k"),
                        axis=mybir.AxisListType.X, op=AluOp.max,
                    )
                    for xt_o, osl in xt_out(li):
                        dw = osl.stop - osl.start
                        pt = ps_p.tile([dw, P], f32, tag="pt")
                        for ki, (xt_s, psl) in enumerate(xin):
                            nc.tensor.matmul(
                                pt[:], lhsT=wp_t[li][:, osl], rhs=xt_s[psl, tsl],
                                start=(ki == 0), stop=False,
                            )
                        nc.tensor.matmul(
                            pt[:], lhsT=br_t[li][:, osl], rhs=ones_row[:, 0:P],
                            start=False, stop=False,
                        )
                        nc.tensor.matmul(
                            pt[:], lhsT=agg[:, osl], rhs=ident[:],
                            is_transpose=True, start=False, stop=True,
                        )
                        nc.scalar.copy(out=xt_o[0:dw, tsl].bitcast(f32r), in_=pt[:])

                # per-tile: S matmuls -> copy -> pack -> top20 -> gather; tail lags 1 tile
                prev = None
                for t in range(NT):
                    tsl = slice(t * P, (t + 1) * P)
                    s_ps = ps_s.tile([P, N], f32, tag="s")
                    for c in range(4):
                        sl = slice(c * 512, (c + 1) * 512)
                        sdt = f32r if F32R_S else f32
                        for ki, (xt_s, psl) in enumerate(xin):
                            nc.tensor.matmul(
                                s_ps[:, sl],
                                lhsT=xt_s[psl, tsl].bitcast(sdt),
                                rhs=xt_s[psl, sl].bitcast(sdt),
                                start=(ki == 0), stop=False,
                            )
                        nc.tensor.matmul(
                            s_ps[:, sl],
                            lhsT=ones_row[:, 0:P].bitcast(f32r),
                            rhs=msqrow[:, sl].bitcast(f32r),
                            start=False, stop=False,
                        )
                        nc.tensor.matmul(
                            s_ps[:, sl],
                            lhsT=msqrow[:, tsl].bitcast(f32r),
                            rhs=ones_row[:, 0:512].bitcast(f32r),
                            start=False, stop=True,
                        )
                    # PSUM->SBUF copy of S rounded to f32r: clears the low 12
                    # mantissa bits, making room for an 11-bit column index.
                    s_sb = work.tile([P, N], f32, tag="s_sb")
                    nc.scalar.copy(out=s_sb[:].bitcast(f32r), in_=s_ps[:])

                    # pack: S_rounded | col_idx  (bitwise ops are DVE-only)
                    spack = packp.tile([P, N], i32, tag="spack")
                    sp2 = packp.tile([P, N], i32, tag="sp2")
                    nc.vector.scalar_tensor_tensor(
                        out=spack[:], in0=s_sb[:].bitcast(i32),
                        scalar=mask_col[:, 0:1], in1=iota_t[:],
                        op0=AluOp.bitwise_and, op1=AluOp.bitwise_or,
                    )

                    m8 = work.tile([P, 24], f32, tag="m8")
                    nc.vector.max(out=m8[:, 0:8], in_=spack[:].bitcast(f32))
                    nc.vector.match_replace(
                        out=sp2[:].bitcast(f32), in_to_replace=m8[:, 0:8],
                        in_values=spack[:].bitcast(f32), imm_value=-3.0e38,
                    )
                    nc.vector.max(out=m8[:, 8:16], in_=sp2[:].bitcast(f32))
                    nc.vector.match_replace(
                        out=spack[:].bitcast(f32), in_to_replace=m8[:, 8:16],
                        in_values=sp2[:].bitcast(f32), imm_value=-3.0e38,
                    )
                    nc.vector.max(out=m8[:, 16:24], in_=spack[:].bitcast(f32))

                    idx = work.tile([P, 24], i32, tag="idx")
                    nc.vector.tensor_scalar(
                        idx[:], m8[:].bitcast(i32), 2047, scalar2=None,
                        op0=AluOp.bitwise_and,
                    )

                    qg = qgp.tile([P, K, do], f32, tag="qg")
                    if GATHER_MODE == "indirect":
                        for s in range(K):
                            nc.gpsimd.indirect_dma_start(
                                out=qg[:, s, :], out_offset=None, in_=q_dram[:],
                                in_offset=bass.IndirectOffsetOnAxis(
                                    ap=idx[:, s : s + 1], axis=0
                                ),
                            )
                    else:
                        # int16 indices, wrapped [16, 160] replicated to 128 partitions
                        idx16 = work.tile([P, K], mybir.dt.int16, tag="idx16")
                        nc.vector.tensor_copy(idx16[:], idx[:, 0:K])
                        idxw = work.tile([P, 8 * K], mybir.dt.int16, tag="idxw")
                        for pq in range(8):
                            nc.sync.dma_start(
                                out=idxw[0:16, pq :: 8].unsqueeze(-1),
                                in_=idx16[16 * pq : 16 * (pq + 1), :].unsqueeze(-1),
                            )
                        for g in range(1, 8):
                            nc.sync.dma_start(
                                out=idxw[16 * g : 16 * (g + 1), :], in_=idxw[0:16, :]
                            )
                        nc.gpsimd.dma_gather(
                            out_ap=qg[:], in_ap=q_dram[:], idxs_ap=idxw[:],
                            num_idxs=K * P, num_idxs_reg=K * P, elem_size=do,
                        )

                    if prev is not None:
                        emit_tail(*prev)
                    prev = (t, qg)
                emit_tail(*prev)

        if dbg is not None:
            for nm, t_ in [("dbg_x1", x1T), ("dbg_x2", x2T), ("dbg_x3", x3T),
                           ("dbg_x4a", x4Ta), ("dbg_x4b", x4Tb)]:
                nc.sync.dma_start(out=dbg[nm][:], in_=t_[:])

        # ---------------- lin1 + global max pool ----------------
        lin_chunks = [
            (x1T, slice(0, 64), slice(0, 64)),
            (x2T, slice(0, 64), slice(64, 128)),
            (x3T, slice(0, 128), slice(128, 256)),
            (x4Ta, slice(0, 128), slice(256, 384)),
            (x4Tb, slice(0, 128), slice(384, 512)),
        ]
        with ExitStack() as tctx:
            tail = tctx.enter_context(tc.tile_pool(name="tail", bufs=1))
            tailw = tctx.enter_context(tc.tile_pool(name="tailw", bufs=2))
            ps_t = tctx.enter_context(tc.tile_pool(name="ps_t", bufs=2, space="PSUM"))

            # lin1 weights as per-K-chunk tiles, all base partition 0
            lw_tiles = []
            for _, xsl, kk in lin_chunks:
                nk = kk.stop - kk.start
                raw = tail.tile([nk, LIN1], f32, tag=f"lwr{kk.start}")
                nc.sync.dma_start(out=raw[:], in_=lw[kk, :])
                t_ = tail.tile([nk, LIN1], f32, tag=f"lw{kk.start}")
                nc.scalar.copy(out=t_[:].bitcast(f32r), in_=raw[:])
                lw_tiles.append(t_)
            lb_sb = tail.tile([P, 8], f32)
            nc.sync.dma_start(out=lb_sb[:], in_=lb[:].rearrange("(t p) -> p t", p=P))

            g_all = tail.tile([P, 8], f32)
            for ct in range(8):
                csl = slice(ct * P, (ct + 1) * P)
                gcols = tailw.tile([P, 4], f32, tag="gcols")
                for c in range(4):
                    hps = ps_t.tile([P, 512], f32, tag="h")
                    psl = slice(c * 512, (c + 1) * 512)
                    ldt = f32r if F32R_LIN1 else f32
                    for ki, (xt_s, xsl, kk) in enumerate(lin_chunks):
                        nc.tensor.matmul(
                            hps[:],
                            lhsT=lw_tiles[ki][:, csl].bitcast(ldt),
                            rhs=xt_s[xsl, psl].bitcast(ldt),
                            start=(ki == 0), stop=(ki == len(lin_chunks) - 1),
                        )
                    nc.vector.tensor_reduce(
                        out=gcols[:, c : c + 1], in_=hps[:],
                        axis=mybir.AxisListType.X, op=AluOp.max,
                    )
                nc.vector.tensor_reduce(
                    out=g_all[:, ct : ct + 1], in_=gcols[:],
                    axis=mybir.AxisListType.X, op=AluOp.max,
                )
            # + lb
            nc.vector.tensor_add(g_all[:], g_all[:], lb_sb[:])
            if dbg is not None:
                nc.sync.dma_start(out=dbg["dbg_g"][:], in_=g_all[:])

            # ---------------- head MLP + log_softmax ----------------
            ones_1 = tail.tile([1, 1], f32)
            nc.vector.memset(ones_1[:], 1.0)

            def head_layer(hi, src_cols, relu):
                # src_cols: [128, nk] tile whose columns are K-chunks of the input
                di, do = HEAD[hi]
                nk = di // P
                wt = tailw.tile([P, nk, do], f32, tag=f"hw{hi}")
                nc.sync.dma_start(
                    out=wt[:],
                    in_=wts[f"m{hi + 1}w"][:].rearrange("(t p) c -> p t c", p=P),
                )
                brow = tailw.tile([1, do], f32, tag=f"hb{hi}")
                nc.sync.dma_start(out=brow[:], in_=wts[f"m{hi + 1}b"][None, :])
                ops = ps_t.tile([1, do], f32, tag="hps")
                for t in range(nk):
                    nc.tensor.matmul(
                        ops[:], lhsT=src_cols[:, t : t + 1], rhs=wt[:, t, :],
                        start=(t == 0), stop=False,
                    )
                nc.tensor.matmul(
                    ops[:], lhsT=ones_1[:], rhs=brow[:], start=False, stop=True
                )
                o_sb = tailw.tile([1, do], f32, tag=f"ho{hi}")
                if relu:
                    nc.scalar.activation(out=o_sb[:], in_=ops[:], func=Act.Relu)
                else:
                    nc.scalar.copy(out=o_sb[:], in_=ops[:])
                if not relu:
                    return o_sb, None
                # transpose to column chunks [128, do//128]
                cols = tailw.tile([P, do // P], f32, tag=f"hc{hi}")
                for t in range(do // P):
                    tp = ps_t.tile([P, 1], f32, tag="htp")
                    # row -> column: out[m,0] = row[0,m] via K=1 matmul with [[1.0]]
                    nc.tensor.matmul(
                        tp[:], lhsT=o_sb[:, t * P : (t + 1) * P], rhs=ones_1[:],
                        start=True, stop=True,
                    )
                    nc.scalar.copy(out=cols[:, t : t + 1], in_=tp[:])
                return o_sb, cols

            _, c1 = head_layer(0, g_all[:], relu=True)
            _, c2 = head_layer(1, c1[:], relu=True)
            logits, _ = head_layer(2, c2[:], relu=False)

            # log_softmax over 10 classes
            mx = tailw.tile([1, 1], f32, tag="mx")
            nc.vector.tensor_reduce(
                out=mx[:], in_=logits[:], axis=mybir.AxisListType.X, op=AluOp.max
            )
            nmx = tailw.tile([1, 1], f32, tag="nmx")
            nc.vector.tensor_scalar_mul(nmx[:], mx[:], -1.0)
            ex = tailw.tile([1, 10], f32, tag="ex")
            se = tailw.tile([1, 1], f32, tag="se")
            nc.scalar.activation(
                out=ex[:], in_=logits[:], func=Act.Exp, bias=nmx[:, 0:1], scale=1.0,
                accum_out=se[:],
            )
            lse = tailw.tile([1, 1], f32, tag="lse")
            nc.scalar.activation(out=lse[:], in_=se[:], func=Act.Ln)
            res = tailw.tile([1, 10], f32, tag="res")
            # res = (logits + nmx) - lse
            nc.vector.scalar_tensor_tensor(
                out=res[:], in0=logits[:], scalar=nmx[:, 0:1],
                in1=lse[:].to_broadcast([1, 10]),
                op0=AluOp.add, op1=AluOp.subtract,
            )
            nc.sync.dma_start(out=out[:], in_=res[:])


def _get_nc():
    if "nc" not in _cache:
        _cache["nc"] = _build()
    return _cache["nc"]


def kernel(**inputs):
    nc = _get_nc()
    np_in = {k: np.asarray(v) for k, v in inputs.items()}
    pos = np_in["pos"].astype(np.float32, copy=False)
    assert pos.shape == (B * N, 3), pos.shape

    names = [f"w{i}" for i in range(1, 5)] + [f"b{i}" for i in range(1, 5)] + [
        "lw", "lb", "m1w", "m1b", "m2w", "m2b", "m3w", "m3b",
    ]
    shared = {k: np.ascontiguousarray(np_in[k], dtype=np.float32) for k in names}
    in_maps = []
    for g in range(B):
        m = dict(shared)
        m["pos"] = np.ascontiguousarray(pos[g * N : (g + 1) * N])
        in_maps.append(m)

    res = run_bass_kernel_spmd(
        nc,
        in_maps,
        core_ids=list(range(B)),
        trace=bool(os.environ.get("KERNEL_TRACE")),
    )
    _cache["last_results"] = res
    return np.concatenate([r["out"] for r in res.results], axis=0)

